# revision 40
# baseline (speedup 1.0000x reference)
"""Trainium2 Bass kernel for 2-head causal self-attention.

Problem: embedded [B=4, S=2048, E=1024], Wq/Wk/Wv [H=2, E, HD=512].
out[b, s, h*HD:(h+1)*HD] = softmax(causal(Q K^T / sqrt(HD))) @ V for head h.

Sharding: 8 (b, h) pairs -> 8 cores, one pair each (perfect SPMD balance).

Per-core dataflow (bf16 operands everywhere; PSUM accumulation f32;
bf16 output, upcast on host):
  - Host passes X^T (so E is on the partition/contraction axis directly).
  - Phase 1: QT[d,q], KT[d,q] (W stationary, X^T moving) and V[k,d]
    (X^T stationary, W moving), q/k pipelined in 512-wide chunks.
    qc=0 runs eo-OUTER with 4 concurrent psum accumulation groups so
    the first matmul needs only the first small wq/xc chunk. DMAs are
    merged into a handful of large triggers (each dma_start costs
    ~0.6us serially on its engine, so dozens of small ones starve the
    pipeline head) issued on TWO parallel streams: weights on Sync,
    x-chunks on Scalar. All pools live for the whole kernel -- closing
    a pool at the phase boundary inserts a drain barrier; one PSUM
    pool with explicit bank tags (q0-3/k0-3) is shared by both phases.
  - Phase 2: scores are computed TRANSPOSED: scoresT[k, q-chunk] =
    (KT tile).T @ QT. After exp, the attnT tile [k, q] is exactly the
    stationary operand needed for ctx[q, d] += attnT.T @ V[k, d] --
    no on-device transpose of the attention matrix is ever needed.
    Softmax denominators come FOR FREE out of the ctx matmuls: V is
    stored interleaved as two 257-wide halves [V[:,0:256] | 1] and
    [V[:,256:512] | 1], so each ctx accumulation also accumulates
    sum_k attnT[k,q] in its column 256 (a psum bank holds only 512
    f32, so a single 513-wide matmul cannot work). This removes the
    old ones-stationary row-sum matmuls (~18k PE cycles) and the
    rotate matmuls entirely.
    Causal masking is a 0/1 multiply on the 4 diagonal-block patterns
    (host constant); strictly-above-diagonal blocks are skipped, and
    diagonal blocks are trimmed to exact 128-col boundaries (bf16
    matmuls run 1 cyc/row at any width, unlike fp32r).
    Phase 2 is software-pipelined: scores(M+1) is emitted before
    ctx(M) so ctx never waits on the exp+mask chain.
NOTE (hard-won): do NOT split qt/kt into per-chunk tiles -- that
layout reproducibly slowed every instruction on the device by ~1.2x.
fp8 is numerically infeasible (measured 6.3e-2 output error vs the
2e-2 gate), so the bf16 PE roofline (~137.5us/core) is the floor.
"""

import contextlib

import ml_dtypes
import numpy as np

import concourse.bass as bass
import concourse.mybir as mybir
from concourse import bacc
import concourse.tile as tile
from concourse import bass_utils

B, S, E, H, HD = 4, 2048, 1024, 2, 512
P = 128
EO = E // P          # 8 e-tiles (contraction for QKV)
DT = HD // P         # 4 d-tiles (contraction for scores)
NKT = S // P         # 16 k-tiles
NSUP = S // 512      # 4 q super-tiles (512 wide)
HH = HD // 2         # 256: half of head dim (ctx computed in 257-wide halves)
SCALE = float(HD) ** -0.5
F32 = mybir.dt.float32
EXP = mybir.ActivationFunctionType.Exp
BF16 = mybir.dt.bfloat16

_NC = None


def _body(tc, xt_d, wq_d, wk_d, wv_d, mask_d, out_d):
    nc = tc.nc

    with contextlib.ExitStack() as ctx:
        per = ctx.enter_context(tc.tile_pool(name="per", bufs=1))
        # Persistent SBUF: QT/KT as [d_inner=128, d_tile, q]; V interleaved as
        # [k_inner, k_tile, half, 257] with a ones column at [..., 256].
        qt = per.tile([P, DT, S], BF16)
        kt = per.tile([P, DT, S], BF16)
        # V split into one tile per 4-k-tile group so phase-2 ctx matmuls on
        # early k-tiles never wait on late phase-1 V writes (coarse per-tile
        # dependency tracking).
        vg = [per.tile([P, 4, 2, HH + 1], BF16, name=f"v{g}") for g in range(4)]
        mask_sb = per.tile([P, 4, 512], BF16)

        # All pools live for the whole kernel: closing a pool at the phase
        # boundary inserts a drain barrier that stalls the first phase-2
        # matmul until every phase-1 consumer has completed. Instead one
        # PSUM pool carries explicit bank tags (q0-3/k0-3) that both phases
        # share; WAR hazards are then tracked per bank, not per pool.
        wpool = ctx.enter_context(tc.tile_pool(name="wpool", bufs=1))
        xpool = ctx.enter_context(tc.tile_pool(name="xpool", bufs=2))
        psu = ctx.enter_context(tc.tile_pool(name="psu", bufs=1, space="PSUM"))
        apool = ctx.enter_context(tc.tile_pool(name="apool", bufs=2))
        opool = ctx.enter_context(tc.tile_pool(name="opool", bufs=3))

        # ---------------- Phase 1: QT, KT, V projections ----------------
        if True:
            wq_sb = wpool.tile([P, EO, HD], BF16)
            wk_sb = wpool.tile([P, EO, HD], BF16)
            wv_sb = wpool.tile([P, EO, HD], BF16)
            xt_r = xt_d.rearrange("(eo p) q -> p eo q", p=P)
            wq_r = wq_d.rearrange("(eo p) d -> p eo d", p=P)
            wk_r = wk_d.rearrange("(eo p) d -> p eo d", p=P)
            wv_r = wv_d.rearrange("(eo p) d -> p eo d", p=P)

            # PE p-state warmup: the tensor engine runs at ~half clock until
            # it has executed ~3us continuously. Burn that ramp on junk
            # matmuls (uninitialized SBUF in, scratch psum out, never read)
            # DURING the initial DMA wait, so the first real matmuls run at
            # full speed. 7 x 512-wide ~= 3us at the ramp clock, ending just
            # before the first input chunk lands.
            warm = psu.tile([P, 512], F32, tag="k3", name="warm")
            for _ in range(7):
                nc.tensor.matmul(
                    warm, lhsT=qt[:, 0, 0:P], rhs=qt[:, 0, 0:512],
                    start=True, stop=True,
                )

            xc0 = xpool.tile([P, EO, 512], BF16, tag="xc", name="xc")
            # Head of the pipeline: wq/xc0 in interleaved 2-eo chunks. The
            # qc=0 projection loops run eo-OUTER with 4 concurrent psum
            # groups, so the first matmul needs only the first 2-eo chunk
            # (512KB) and consumption (0.85us/eo) stays below steady-state
            # DMA supply (~0.77us/eo). Later tensors have slack and go out
            # as big merged triggers.
            # Two parallel trigger streams: sync issues the weights, scalar
            # issues the x chunks, so the first wq and xc0 transfers start
            # simultaneously instead of serializing on one engine.
            # Finest chunks first (the first matmul needs only eo=0 of each),
            # then pairs -- supply rate stays ahead of the eo-outer consumers.
            head_chunks = [(0, 1), (1, 2), (2, 3), (3, 4), (4, 6), (6, 8)]
            for a, b in head_chunks:
                nc.sync.dma_start(
                    out=wq_sb[:, a:b, :], in_=wq_r[:, a:b, :]
                )
                nc.scalar.dma_start(
                    out=xc0[:, a:b, :], in_=xt_r[:, a:b, 0:512]
                )
            nc.sync.dma_start(out=wk_sb[:, 0:4, :], in_=wk_r[:, 0:4, :])
            nc.sync.dma_start(out=wk_sb[:, 4:8, :], in_=wk_r[:, 4:8, :])
            nc.sync.dma_start(out=wv_sb, in_=wv_r)
            nc.sync.dma_start(out=mask_sb, in_=mask_d.rearrange("r p c -> p r c"))
            # V's ones columns: emitted after the triggers so the gpsimd ops
            # don't delay the preamble handoff that gates the first trigger.
            for g in vg:
                nc.gpsimd.memset(g[:, :, :, HH], 1.0)

            for qc in range(4):  # 512-wide q/k chunk
                if qc == 0:
                    xc = xc0
                else:
                    xc = xpool.tile([P, EO, 512], BF16, tag="xc", name="xc")
                    eng = nc.scalar if qc == 1 else nc.sync
                    eng.dma_start(
                        out=xc, in_=xt_r[:, :, qc * 512 : (qc + 1) * 512]
                    )

                # QT / KT: out[d_tile, q-chunk] = sum_e W[e, d].T @ XT[e, q].
                # eo-outer: 4 dm accumulation groups open at once (4 banks),
                # each eo chunk is consumed as soon as it lands. PSUM->SBUF
                # casts alternate scalar/vector so neither serializes the
                # bank handoff to the next projection.
                for w_sb, dst, tg in ((wq_sb, qt, "q"), (wk_sb, kt, "k")):
                    pss4 = [
                        psu.tile([P, 512], F32, tag=f"{tg}{dm}", name=f"p{tg}{dm}")
                        for dm in range(DT)
                    ]
                    for eo in range(EO):
                        for dm in range(DT):
                            nc.tensor.matmul(
                                pss4[dm],
                                lhsT=w_sb[:, eo, dm * P : (dm + 1) * P],
                                rhs=xc[:, eo, :],
                                start=(eo == 0),
                                stop=(eo == EO - 1),
                            )
                    for dm in range(DT):
                        dslice = dst[:, dm, qc * 512 : (qc + 1) * 512]
                        if dm % 2 == 0:
                            nc.scalar.copy(dslice, pss4[dm])
                        else:
                            nc.vector.tensor_copy(dslice, pss4[dm])

                # V: out[k_tile, d] = sum_e XT[e, k].T @ Wv[e, d], stored as
                # two 256-wide halves (ones columns already set).
                for ki in range(4):
                    kg = qc * 4 + ki
                    ps = psu.tile([P, 512], F32, tag=f"q{ki}", name="psv")
                    for eo in range(EO):
                        nc.tensor.matmul(
                            ps,
                            lhsT=xc[:, eo, ki * P : (ki + 1) * P],
                            rhs=wv_sb[:, eo, :],
                            start=(eo == 0),
                            stop=(eo == EO - 1),
                        )
                    nc.vector.tensor_copy(vg[qc][:, ki, 0, 0:HH], ps[:, 0:HH])
                    nc.vector.tensor_copy(vg[qc][:, ki, 1, 0:HH], ps[:, HH:HD])

        # ---------------- Phase 2: attention ----------------
        # Scores rotate through KT's four banks (k0-3, drained earliest in
        # phase 1); ctx accumulators take QT/V's banks (q0-3).
        if True:
            def emit_scores(M):
                # One at tile per k-tile so a ctx matmul on k-tile j waits
                # only for j's own exp+mask, not the whole super-tile's.
                njt = 4 * M + 4  # causal: k-tiles 0 .. 4M+3
                at = [
                    apool.tile([P, 512], BF16, tag=f"at{j}", name=f"at{j}")
                    for j in range(njt)
                ]
                for j in range(njt):
                    r = j - 4 * M
                    # Diagonal-zone tiles: cols < 128r are fully masked; bf16
                    # matmuls have no minimum-width penalty, so trim exactly.
                    off = P * r if r > 0 else 0
                    ps = psu.tile([P, 512], F32, tag=f"k{j % 4}", name="ps_s")
                    for dt_i in range(DT):
                        nc.tensor.matmul(
                            ps[:, off:512],
                            lhsT=kt[:, dt_i, j * P : (j + 1) * P],
                            rhs=qt[:, dt_i, M * 512 + off : (M + 1) * 512],
                            start=(dt_i == 0),
                            stop=(dt_i == DT - 1),
                        )
                    a_j = at[j][:, off:512]
                    # attnT[k, q] = exp(scoresT / sqrt(hd)); masking after.
                    nc.scalar.activation(a_j, ps[:, off:512], EXP, scale=SCALE)
                    if r >= 0:  # diagonal-zone: zero invalid (q < k) cols
                        nc.vector.tensor_mul(a_j, a_j, mask_sb[:, r, off:512])
                return at

            def emit_ctx(M, at, final=False):
                # ctx[q_sub, d] += attnT_tile.T @ V-half (column 256 of each
                # psum accumulates the softmax denominator via V's ones
                # column). NOTE: hardware start=True invalidates has_written
                # for the WHOLE psum bank, so only one accumulation group may
                # be open per bank at a time; 4 banks = 2 subtiles in flight,
                # so process s in two waves, reusing bank tags across waves.
                for wave in ((3, 2), (1, 0)):
                    wtiles = {}
                    for wi, s in enumerate(wave):
                        cps = [
                            psu.tile(
                                [P, 512], F32, tag=f"q{2 * wi + h}", name=f"c{wi}{h}"
                            )
                            for h in (0, 1)
                        ]
                        wtiles[s] = cps
                        nj = 4 * M + s + 1
                        if final and s == 0:
                            # Very last chain: run h0 (which carries the
                            # denominator column) to completion FIRST, so its
                            # reciprocal + scale + store overlap the h1 chain
                            # and only h1's short store tail remains exposed.
                            for h in (0, 1):
                                for j in range(nj):
                                    nc.tensor.matmul(
                                        cps[h][:, 0 : HH + 1],
                                        lhsT=at[j][:, s * P : (s + 1) * P],
                                        rhs=vg[j // 4][:, j % 4, h, :],
                                        start=(j == 0),
                                        stop=(j == nj - 1),
                                    )
                                if h == 0:
                                    rinv0 = opool.tile([P, 1], F32, tag="rinv")
                                    nc.vector.reciprocal(
                                        rinv0, cps[0][:, HH : HH + 1]
                                    )
                                    o0 = opool.tile([P, HH], BF16, tag="o0", name="o0")
                                    nc.scalar.mul(o0, cps[0][:, 0:HH], rinv0)
                                    row0 = M * 512
                                    nc.sync.dma_start(
                                        out=out_d[row0 : row0 + P, 0:HH], in_=o0
                                    )
                            o1 = opool.tile([P, HH], BF16, tag="o1", name="o1")
                            nc.vector.tensor_scalar_mul(o1, cps[1][:, 0:HH], rinv0)
                            nc.scalar.dma_start(
                                out=out_d[row0 : row0 + P, HH:HD], in_=o1
                            )
                            continue
                        for j in range(nj):
                            a_st = at[j][:, s * P : (s + 1) * P]
                            for h in (0, 1):
                                nc.tensor.matmul(
                                    cps[h][:, 0 : HH + 1],
                                    lhsT=a_st,
                                    rhs=vg[j // 4][:, j % 4, h, :],
                                    start=(j == 0),
                                    stop=(j == nj - 1),
                                )
                    for s in wave:
                        if final and s == 0:
                            continue
                        cps = wtiles[s]
                        rinv = opool.tile([P, 1], F32, tag="rinv")
                        nc.vector.reciprocal(rinv, cps[0][:, HH : HH + 1])
                        row0 = M * 512 + s * P
                        # Per-half output on two engines in parallel (scalar
                        # scales+triggers h0, vector h1) so the final store
                        # chain after the last matmul is as short as possible.
                        nq = 2
                        w = HD // nq
                        for qi in range(nq):
                            # bf16 store: halves DVE scale time and DMA bytes;
                            # host upcasts. Error budget absorbs it (~0.4%).
                            oq = opool.tile([P, w], BF16, tag=f"o{qi}", name="oq")
                            c0 = qi * w
                            src_ap = cps[c0 // HH]
                            lo = c0 % HH
                            if qi % 2 == 0:
                                nc.scalar.mul(oq, src_ap[:, lo : lo + w], rinv)
                                nc.sync.dma_start(
                                    out=out_d[row0 : row0 + P, c0 : c0 + w], in_=oq
                                )
                            else:
                                nc.vector.tensor_scalar_mul(
                                    oq, src_ap[:, lo : lo + w], rinv
                                )
                                nc.scalar.dma_start(
                                    out=out_d[row0 : row0 + P, c0 : c0 + w], in_=oq
                                )

            # Software-pipeline: emit scores(M+1) before ctx(M) so every ctx
            # chain has a full super-tile of scores runway between a diagonal
            # tile's exp+mask and the ctx matmuls that consume it (at M=0 the
            # scores alone are too short to hide the scalar/vector latency).
            # at-tile tags alternate between 2 buffers, which exactly matches
            # this interleave depth.
            prev = None
            for M in range(NSUP):  # q super-tile: q in [512M, 512(M+1))
                at = emit_scores(M)
                if prev is not None:
                    emit_ctx(M - 1, prev)
                prev = at
            emit_ctx(NSUP - 1, prev, final=True)


def _build_nc():
    nc = bacc.Bacc("TRN2", target_bir_lowering=False, debug=False, num_devices=8)
    xt_d = nc.dram_tensor("xt", [E, S], BF16, kind="ExternalInput")
    wq_d = nc.dram_tensor("wq", [E, HD], BF16, kind="ExternalInput")
    wk_d = nc.dram_tensor("wk", [E, HD], BF16, kind="ExternalInput")
    wv_d = nc.dram_tensor("wv", [E, HD], BF16, kind="ExternalInput")
    mask_d = nc.dram_tensor("mask", [4, P, 512], BF16, kind="ExternalInput")
    out_d = nc.dram_tensor("out", [S, HD], BF16, kind="ExternalOutput")
    with tile.TileContext(nc) as tc:
        _body(tc, xt_d.ap(), wq_d.ap(), wk_d.ap(), wv_d.ap(), mask_d.ap(), out_d.ap())
    nc.compile()
    return nc


def _mask_np():
    # mask[r][k_local, q_local] = 1 iff q_local >= 128*r + k_local
    q = np.arange(512)[None, :]
    k = np.arange(P)[:, None]
    return np.stack(
        [(q >= (P * r + k)).astype(np.float32) for r in range(4)], axis=0
    ).astype(ml_dtypes.bfloat16)


def _in_maps(embedded, Wq, Wk, Wv):
    embedded = np.asarray(embedded, dtype=np.float32)
    Wq = np.asarray(Wq, dtype=np.float32)
    Wk = np.asarray(Wk, dtype=np.float32)
    Wv = np.asarray(Wv, dtype=np.float32)
    mask = _mask_np()
    in_maps = []
    for core in range(8):
        b, h = divmod(core, 2)
        in_maps.append(
            {
                "xt": np.ascontiguousarray(embedded[b].T).astype(ml_dtypes.bfloat16),
                "wq": np.ascontiguousarray(Wq[h]).astype(ml_dtypes.bfloat16),
                "wk": np.ascontiguousarray(Wk[h]).astype(ml_dtypes.bfloat16),
                "wv": np.ascontiguousarray(Wv[h]).astype(ml_dtypes.bfloat16),
                "mask": mask,
            }
        )
    return in_maps


def _gather(results):
    out = np.empty((B, S, H * HD), np.float32)
    for core in range(8):
        b, h = divmod(core, 2)
        out[b, :, h * HD : (h + 1) * HD] = results[core]["out"].astype(np.float32)
    return out


def _get_nc():
    global _NC
    if _NC is None:
        _NC = _build_nc()
    return _NC


def kernel(embedded, Wq, Wk, Wv):
    res = bass_utils.run_bass_kernel_spmd(
        _get_nc(), _in_maps(embedded, Wq, Wk, Wv), core_ids=list(range(8))
    )
    return _gather(res.results)


def kernel_traced(embedded, Wq, Wk, Wv):
    """Like kernel() but with NTFF tracing; returns (out, BassKernelResults)."""
    res = bass_utils.run_bass_kernel_spmd(
        _get_nc(), _in_maps(embedded, Wq, Wk, Wv), core_ids=list(range(8)), trace=True
    )
    return _gather(res.results), res


# revision 41
# speedup vs baseline: 1.0041x; 1.0041x over previous
"""Trainium2 Bass kernel for 2-head causal self-attention.

Problem: embedded [B=4, S=2048, E=1024], Wq/Wk/Wv [H=2, E, HD=512].
out[b, s, h*HD:(h+1)*HD] = softmax(causal(Q K^T / sqrt(HD))) @ V for head h.

Sharding: 8 (b, h) pairs -> 8 cores, one pair each (perfect SPMD balance).

Per-core dataflow (bf16 operands everywhere; PSUM accumulation f32;
bf16 output, upcast on host):
  - Host passes X^T (so E is on the partition/contraction axis directly).
  - Phase 1: QT[d,q], KT[d,q] (W stationary, X^T moving) and V[k,d]
    (X^T stationary, W moving), q/k pipelined in 512-wide chunks.
    qc=0 runs eo-OUTER with 4 concurrent psum accumulation groups so
    the first matmul needs only the first small wq/xc chunk. DMAs are
    merged into a handful of large triggers (each dma_start costs
    ~0.6us serially on its engine, so dozens of small ones starve the
    pipeline head) issued on TWO parallel streams: weights on Sync,
    x-chunks on Scalar. All pools live for the whole kernel -- closing
    a pool at the phase boundary inserts a drain barrier; one PSUM
    pool with explicit bank tags (q0-3/k0-3) is shared by both phases.
  - Phase 2: scores are computed TRANSPOSED: scoresT[k, q-chunk] =
    (KT tile).T @ QT. After exp, the attnT tile [k, q] is exactly the
    stationary operand needed for ctx[q, d] += attnT.T @ V[k, d] --
    no on-device transpose of the attention matrix is ever needed.
    Softmax denominators come FOR FREE out of the ctx matmuls: V is
    stored interleaved as two 257-wide halves [V[:,0:256] | 1] and
    [V[:,256:512] | 1], so each ctx accumulation also accumulates
    sum_k attnT[k,q] in its column 256 (a psum bank holds only 512
    f32, so a single 513-wide matmul cannot work). This removes the
    old ones-stationary row-sum matmuls (~18k PE cycles) and the
    rotate matmuls entirely.
    Causal masking is a 0/1 multiply on the 4 diagonal-block patterns
    (host constant); strictly-above-diagonal blocks are skipped, and
    diagonal blocks are trimmed to exact 128-col boundaries (bf16
    matmuls run 1 cyc/row at any width, unlike fp32r).
    Phase 2 is software-pipelined: scores(M+1) is emitted before
    ctx(M) so ctx never waits on the exp+mask chain.
NOTE (hard-won): do NOT split qt/kt into per-chunk tiles -- that
layout reproducibly slowed every instruction on the device by ~1.2x.
fp8 is numerically infeasible (measured 6.3e-2 output error vs the
2e-2 gate), so the bf16 PE roofline (~137.5us/core) is the floor.
"""

import contextlib

import ml_dtypes
import numpy as np

import concourse.bass as bass
import concourse.mybir as mybir
from concourse import bacc
import concourse.tile as tile
from concourse import bass_utils

B, S, E, H, HD = 4, 2048, 1024, 2, 512
P = 128
EO = E // P          # 8 e-tiles (contraction for QKV)
DT = HD // P         # 4 d-tiles (contraction for scores)
NKT = S // P         # 16 k-tiles
NSUP = S // 512      # 4 q super-tiles (512 wide)
HH = HD // 2         # 256: half of head dim (ctx computed in 257-wide halves)
SCALE = float(HD) ** -0.5
F32 = mybir.dt.float32
EXP = mybir.ActivationFunctionType.Exp
BF16 = mybir.dt.bfloat16

_NC = None


def _body(tc, xt_d, wq_d, wk_d, wv_d, mask_d, out_d):
    nc = tc.nc

    with contextlib.ExitStack() as ctx:
        per = ctx.enter_context(tc.tile_pool(name="per", bufs=1))
        # Persistent SBUF: QT/KT as [d_inner=128, d_tile, q]; V interleaved as
        # [k_inner, k_tile, half, 257] with a ones column at [..., 256].
        qt = per.tile([P, DT, S], BF16)
        kt = per.tile([P, DT, S], BF16)
        # V split into one tile per 4-k-tile group so phase-2 ctx matmuls on
        # early k-tiles never wait on late phase-1 V writes (coarse per-tile
        # dependency tracking).
        vg = [per.tile([P, 4, 2, HH + 1], BF16, name=f"v{g}") for g in range(4)]
        mask_sb = per.tile([P, 4, 512], BF16)

        # All pools live for the whole kernel: closing a pool at the phase
        # boundary inserts a drain barrier that stalls the first phase-2
        # matmul until every phase-1 consumer has completed. Instead one
        # PSUM pool carries explicit bank tags (q0-3/k0-3) that both phases
        # share; WAR hazards are then tracked per bank, not per pool.
        wpool = ctx.enter_context(tc.tile_pool(name="wpool", bufs=1))
        xpool = ctx.enter_context(tc.tile_pool(name="xpool", bufs=2))
        psu = ctx.enter_context(tc.tile_pool(name="psu", bufs=1, space="PSUM"))
        apool = ctx.enter_context(tc.tile_pool(name="apool", bufs=2))
        opool = ctx.enter_context(tc.tile_pool(name="opool", bufs=3))

        # ---------------- Phase 1: QT, KT, V projections ----------------
        if True:
            wq_sb = wpool.tile([P, EO, HD], BF16)
            wk_sb = wpool.tile([P, EO, HD], BF16)
            wv_sb = wpool.tile([P, EO, HD], BF16)
            xt_r = xt_d.rearrange("(eo p) q -> p eo q", p=P)
            wq_r = wq_d.rearrange("(eo p) d -> p eo d", p=P)
            wk_r = wk_d.rearrange("(eo p) d -> p eo d", p=P)
            wv_r = wv_d.rearrange("(eo p) d -> p eo d", p=P)

            # PE p-state warmup: the tensor engine runs at ~half clock until
            # it has executed ~3us continuously. Burn that ramp on junk
            # matmuls (uninitialized SBUF in, scratch psum out, never read)
            # DURING the initial DMA wait, so the first real matmuls run at
            # full speed. 7 x 512-wide ~= 3us at the ramp clock, ending just
            # before the first input chunk lands.
            warm = psu.tile([P, 512], F32, tag="k3", name="warm")
            for _ in range(7):
                nc.tensor.matmul(
                    warm, lhsT=qt[:, 0, 0:P], rhs=qt[:, 0, 0:512],
                    start=True, stop=True,
                )

            xc0 = xpool.tile([P, EO, 512], BF16, tag="xc", name="xc")
            # Head of the pipeline: wq/xc0 in interleaved 2-eo chunks. The
            # qc=0 projection loops run eo-OUTER with 4 concurrent psum
            # groups, so the first matmul needs only the first 2-eo chunk
            # (512KB) and consumption (0.85us/eo) stays below steady-state
            # DMA supply (~0.77us/eo). Later tensors have slack and go out
            # as big merged triggers.
            # Two parallel trigger streams: sync issues the weights, scalar
            # issues the x chunks, so the first wq and xc0 transfers start
            # simultaneously instead of serializing on one engine.
            # Finest chunks first (the first matmul needs only eo=0 of each),
            # then pairs -- supply rate stays ahead of the eo-outer consumers.
            head_chunks = [(0, 1), (1, 2), (2, 4), (4, 6), (6, 8)]
            for a, b in head_chunks:
                nc.sync.dma_start(
                    out=wq_sb[:, a:b, :], in_=wq_r[:, a:b, :]
                )
                nc.scalar.dma_start(
                    out=xc0[:, a:b, :], in_=xt_r[:, a:b, 0:512]
                )
            nc.sync.dma_start(out=wk_sb[:, 0:4, :], in_=wk_r[:, 0:4, :])
            nc.sync.dma_start(out=wk_sb[:, 4:8, :], in_=wk_r[:, 4:8, :])
            nc.sync.dma_start(out=wv_sb, in_=wv_r)
            nc.sync.dma_start(out=mask_sb, in_=mask_d.rearrange("r p c -> p r c"))
            # V's ones columns: emitted after the triggers so the gpsimd ops
            # don't delay the preamble handoff that gates the first trigger.
            for g in vg:
                nc.gpsimd.memset(g[:, :, :, HH], 1.0)

            for qc in range(4):  # 512-wide q/k chunk
                if qc == 0:
                    xc = xc0
                else:
                    xc = xpool.tile([P, EO, 512], BF16, tag="xc", name="xc")
                    eng = nc.scalar if qc == 1 else nc.sync
                    eng.dma_start(
                        out=xc, in_=xt_r[:, :, qc * 512 : (qc + 1) * 512]
                    )

                # QT / KT: out[d_tile, q-chunk] = sum_e W[e, d].T @ XT[e, q].
                # eo-outer: 4 dm accumulation groups open at once (4 banks),
                # each eo chunk is consumed as soon as it lands. PSUM->SBUF
                # casts alternate scalar/vector so neither serializes the
                # bank handoff to the next projection.
                for w_sb, dst, tg in ((wq_sb, qt, "q"), (wk_sb, kt, "k")):
                    pss4 = [
                        psu.tile([P, 512], F32, tag=f"{tg}{dm}", name=f"p{tg}{dm}")
                        for dm in range(DT)
                    ]
                    for eo in range(EO):
                        for dm in range(DT):
                            nc.tensor.matmul(
                                pss4[dm],
                                lhsT=w_sb[:, eo, dm * P : (dm + 1) * P],
                                rhs=xc[:, eo, :],
                                start=(eo == 0),
                                stop=(eo == EO - 1),
                            )
                    for dm in range(DT):
                        dslice = dst[:, dm, qc * 512 : (qc + 1) * 512]
                        if dm % 2 == 0:
                            nc.scalar.copy(dslice, pss4[dm])
                        else:
                            nc.vector.tensor_copy(dslice, pss4[dm])

                # V: out[k_tile, d] = sum_e XT[e, k].T @ Wv[e, d], stored as
                # two 256-wide halves (ones columns already set).
                for ki in range(4):
                    kg = qc * 4 + ki
                    ps = psu.tile([P, 512], F32, tag=f"q{ki}", name="psv")
                    for eo in range(EO):
                        nc.tensor.matmul(
                            ps,
                            lhsT=xc[:, eo, ki * P : (ki + 1) * P],
                            rhs=wv_sb[:, eo, :],
                            start=(eo == 0),
                            stop=(eo == EO - 1),
                        )
                    nc.vector.tensor_copy(vg[qc][:, ki, 0, 0:HH], ps[:, 0:HH])
                    nc.vector.tensor_copy(vg[qc][:, ki, 1, 0:HH], ps[:, HH:HD])

        # ---------------- Phase 2: attention ----------------
        # Scores rotate through KT's four banks (k0-3, drained earliest in
        # phase 1); ctx accumulators take QT/V's banks (q0-3).
        if True:
            def emit_scores(M):
                # One at tile per k-tile so a ctx matmul on k-tile j waits
                # only for j's own exp+mask, not the whole super-tile's.
                njt = 4 * M + 4  # causal: k-tiles 0 .. 4M+3
                at = [
                    apool.tile([P, 512], BF16, tag=f"at{j}", name=f"at{j}")
                    for j in range(njt)
                ]
                for j in range(njt):
                    r = j - 4 * M
                    # Diagonal-zone tiles: cols < 128r are fully masked; bf16
                    # matmuls have no minimum-width penalty, so trim exactly.
                    off = P * r if r > 0 else 0
                    ps = psu.tile([P, 512], F32, tag=f"k{j % 4}", name="ps_s")
                    for dt_i in range(DT):
                        nc.tensor.matmul(
                            ps[:, off:512],
                            lhsT=kt[:, dt_i, j * P : (j + 1) * P],
                            rhs=qt[:, dt_i, M * 512 + off : (M + 1) * 512],
                            start=(dt_i == 0),
                            stop=(dt_i == DT - 1),
                        )
                    a_j = at[j][:, off:512]
                    # attnT[k, q] = exp(scoresT / sqrt(hd)); masking after.
                    nc.scalar.activation(a_j, ps[:, off:512], EXP, scale=SCALE)
                    if r >= 0:  # diagonal-zone: zero invalid (q < k) cols
                        nc.vector.tensor_mul(a_j, a_j, mask_sb[:, r, off:512])
                return at

            def emit_ctx(M, at, final=False):
                # ctx[q_sub, d] += attnT_tile.T @ V-half (column 256 of each
                # psum accumulates the softmax denominator via V's ones
                # column). NOTE: hardware start=True invalidates has_written
                # for the WHOLE psum bank, so only one accumulation group may
                # be open per bank at a time; 4 banks = 2 subtiles in flight,
                # so process s in two waves, reusing bank tags across waves.
                for wave in ((3, 2), (1, 0)):
                    wtiles = {}
                    for wi, s in enumerate(wave):
                        cps = [
                            psu.tile(
                                [P, 512], F32, tag=f"q{2 * wi + h}", name=f"c{wi}{h}"
                            )
                            for h in (0, 1)
                        ]
                        wtiles[s] = cps
                        nj = 4 * M + s + 1
                        if final and s == 0:
                            # Very last chain: run h0 (which carries the
                            # denominator column) to completion FIRST, so its
                            # reciprocal + scale + store overlap the h1 chain
                            # and only h1's short store tail remains exposed.
                            for h in (0, 1):
                                for j in range(nj):
                                    nc.tensor.matmul(
                                        cps[h][:, 0 : HH + 1],
                                        lhsT=at[j][:, s * P : (s + 1) * P],
                                        rhs=vg[j // 4][:, j % 4, h, :],
                                        start=(j == 0),
                                        stop=(j == nj - 1),
                                    )
                                if h == 0:
                                    rinv0 = opool.tile([P, 1], F32, tag="rinv")
                                    nc.vector.reciprocal(
                                        rinv0, cps[0][:, HH : HH + 1]
                                    )
                                    o0 = opool.tile([P, HH], BF16, tag="o0", name="o0")
                                    nc.scalar.mul(o0, cps[0][:, 0:HH], rinv0)
                                    row0 = M * 512
                                    nc.sync.dma_start(
                                        out=out_d[row0 : row0 + P, 0:HH], in_=o0
                                    )
                            o1 = opool.tile([P, HH], BF16, tag="o1", name="o1")
                            nc.vector.tensor_scalar_mul(o1, cps[1][:, 0:HH], rinv0)
                            nc.scalar.dma_start(
                                out=out_d[row0 : row0 + P, HH:HD], in_=o1
                            )
                            continue
                        for j in range(nj):
                            a_st = at[j][:, s * P : (s + 1) * P]
                            for h in (0, 1):
                                nc.tensor.matmul(
                                    cps[h][:, 0 : HH + 1],
                                    lhsT=a_st,
                                    rhs=vg[j // 4][:, j % 4, h, :],
                                    start=(j == 0),
                                    stop=(j == nj - 1),
                                )
                    for s in wave:
                        if final and s == 0:
                            continue
                        cps = wtiles[s]
                        rinv = opool.tile([P, 1], F32, tag="rinv")
                        nc.vector.reciprocal(rinv, cps[0][:, HH : HH + 1])
                        row0 = M * 512 + s * P
                        # Per-half output on two engines in parallel (scalar
                        # scales+triggers h0, vector h1) so the final store
                        # chain after the last matmul is as short as possible.
                        nq = 2
                        w = HD // nq
                        for qi in range(nq):
                            # bf16 store: halves DVE scale time and DMA bytes;
                            # host upcasts. Error budget absorbs it (~0.4%).
                            oq = opool.tile([P, w], BF16, tag=f"o{qi}", name="oq")
                            c0 = qi * w
                            src_ap = cps[c0 // HH]
                            lo = c0 % HH
                            if qi % 2 == 0:
                                nc.scalar.mul(oq, src_ap[:, lo : lo + w], rinv)
                                nc.sync.dma_start(
                                    out=out_d[row0 : row0 + P, c0 : c0 + w], in_=oq
                                )
                            else:
                                nc.vector.tensor_scalar_mul(
                                    oq, src_ap[:, lo : lo + w], rinv
                                )
                                nc.scalar.dma_start(
                                    out=out_d[row0 : row0 + P, c0 : c0 + w], in_=oq
                                )

            # Software-pipeline: emit scores(M+1) before ctx(M) so every ctx
            # chain has a full super-tile of scores runway between a diagonal
            # tile's exp+mask and the ctx matmuls that consume it (at M=0 the
            # scores alone are too short to hide the scalar/vector latency).
            # at-tile tags alternate between 2 buffers, which exactly matches
            # this interleave depth.
            prev = None
            for M in range(NSUP):  # q super-tile: q in [512M, 512(M+1))
                at = emit_scores(M)
                if prev is not None:
                    emit_ctx(M - 1, prev)
                prev = at
            emit_ctx(NSUP - 1, prev, final=True)


def _build_nc():
    nc = bacc.Bacc("TRN2", target_bir_lowering=False, debug=False, num_devices=8)
    xt_d = nc.dram_tensor("xt", [E, S], BF16, kind="ExternalInput")
    wq_d = nc.dram_tensor("wq", [E, HD], BF16, kind="ExternalInput")
    wk_d = nc.dram_tensor("wk", [E, HD], BF16, kind="ExternalInput")
    wv_d = nc.dram_tensor("wv", [E, HD], BF16, kind="ExternalInput")
    mask_d = nc.dram_tensor("mask", [4, P, 512], BF16, kind="ExternalInput")
    out_d = nc.dram_tensor("out", [S, HD], BF16, kind="ExternalOutput")
    with tile.TileContext(nc) as tc:
        _body(tc, xt_d.ap(), wq_d.ap(), wk_d.ap(), wv_d.ap(), mask_d.ap(), out_d.ap())
    nc.compile()
    return nc


def _mask_np():
    # mask[r][k_local, q_local] = 1 iff q_local >= 128*r + k_local
    q = np.arange(512)[None, :]
    k = np.arange(P)[:, None]
    return np.stack(
        [(q >= (P * r + k)).astype(np.float32) for r in range(4)], axis=0
    ).astype(ml_dtypes.bfloat16)


def _in_maps(embedded, Wq, Wk, Wv):
    embedded = np.asarray(embedded, dtype=np.float32)
    Wq = np.asarray(Wq, dtype=np.float32)
    Wk = np.asarray(Wk, dtype=np.float32)
    Wv = np.asarray(Wv, dtype=np.float32)
    mask = _mask_np()
    in_maps = []
    for core in range(8):
        b, h = divmod(core, 2)
        in_maps.append(
            {
                "xt": np.ascontiguousarray(embedded[b].T).astype(ml_dtypes.bfloat16),
                "wq": np.ascontiguousarray(Wq[h]).astype(ml_dtypes.bfloat16),
                "wk": np.ascontiguousarray(Wk[h]).astype(ml_dtypes.bfloat16),
                "wv": np.ascontiguousarray(Wv[h]).astype(ml_dtypes.bfloat16),
                "mask": mask,
            }
        )
    return in_maps


def _gather(results):
    out = np.empty((B, S, H * HD), np.float32)
    for core in range(8):
        b, h = divmod(core, 2)
        out[b, :, h * HD : (h + 1) * HD] = results[core]["out"].astype(np.float32)
    return out


def _get_nc():
    global _NC
    if _NC is None:
        _NC = _build_nc()
    return _NC


def kernel(embedded, Wq, Wk, Wv):
    res = bass_utils.run_bass_kernel_spmd(
        _get_nc(), _in_maps(embedded, Wq, Wk, Wv), core_ids=list(range(8))
    )
    return _gather(res.results)


def kernel_traced(embedded, Wq, Wk, Wv):
    """Like kernel() but with NTFF tracing; returns (out, BassKernelResults)."""
    res = bass_utils.run_bass_kernel_spmd(
        _get_nc(), _in_maps(embedded, Wq, Wk, Wv), core_ids=list(range(8)), trace=True
    )
    return _gather(res.results), res


# revision 42
# speedup vs baseline: 1.0078x; 1.0037x over previous
"""Trainium2 Bass kernel for 2-head causal self-attention.

Problem: embedded [B=4, S=2048, E=1024], Wq/Wk/Wv [H=2, E, HD=512].
out[b, s, h*HD:(h+1)*HD] = softmax(causal(Q K^T / sqrt(HD))) @ V for head h.

Sharding: 8 (b, h) pairs -> 8 cores, one pair each (perfect SPMD balance).

Per-core dataflow (bf16 operands everywhere; PSUM accumulation f32;
bf16 output, upcast on host):
  - Host passes X^T (so E is on the partition/contraction axis directly).
  - Phase 1: QT[d,q], KT[d,q] (W stationary, X^T moving) and V[k,d]
    (X^T stationary, W moving), q/k pipelined in 512-wide chunks.
    qc=0 runs eo-OUTER with 4 concurrent psum accumulation groups so
    the first matmul needs only the first small wq/xc chunk. DMAs are
    merged into a handful of large triggers (each dma_start costs
    ~0.6us serially on its engine, so dozens of small ones starve the
    pipeline head) issued on TWO parallel streams: weights on Sync,
    x-chunks on Scalar. All pools live for the whole kernel -- closing
    a pool at the phase boundary inserts a drain barrier; one PSUM
    pool with explicit bank tags (q0-3/k0-3) is shared by both phases.
  - Phase 2: scores are computed TRANSPOSED: scoresT[k, q-chunk] =
    (KT tile).T @ QT. After exp, the attnT tile [k, q] is exactly the
    stationary operand needed for ctx[q, d] += attnT.T @ V[k, d] --
    no on-device transpose of the attention matrix is ever needed.
    Softmax denominators come FOR FREE out of the ctx matmuls: V is
    stored interleaved as two 257-wide halves [V[:,0:256] | 1] and
    [V[:,256:512] | 1], so each ctx accumulation also accumulates
    sum_k attnT[k,q] in its column 256 (a psum bank holds only 512
    f32, so a single 513-wide matmul cannot work). This removes the
    old ones-stationary row-sum matmuls (~18k PE cycles) and the
    rotate matmuls entirely.
    Causal masking is a 0/1 multiply on the 4 diagonal-block patterns
    (host constant); strictly-above-diagonal blocks are skipped, and
    diagonal blocks are trimmed to exact 128-col boundaries (bf16
    matmuls run 1 cyc/row at any width, unlike fp32r).
    Phase 2 is software-pipelined: scores(M+1) is emitted before
    ctx(M) so ctx never waits on the exp+mask chain.
NOTE (hard-won): do NOT split qt/kt into per-chunk tiles -- that
layout reproducibly slowed every instruction on the device by ~1.2x.
fp8 is numerically infeasible (measured 6.3e-2 output error vs the
2e-2 gate), so the bf16 PE roofline (~137.5us/core) is the floor.
"""

import contextlib

import ml_dtypes
import numpy as np

import concourse.bass as bass
import concourse.mybir as mybir
from concourse import bacc
import concourse.tile as tile
from concourse import bass_utils

B, S, E, H, HD = 4, 2048, 1024, 2, 512
P = 128
EO = E // P          # 8 e-tiles (contraction for QKV)
DT = HD // P         # 4 d-tiles (contraction for scores)
NKT = S // P         # 16 k-tiles
NSUP = S // 512      # 4 q super-tiles (512 wide)
HH = HD // 2         # 256: half of head dim (ctx computed in 257-wide halves)
SCALE = float(HD) ** -0.5
F32 = mybir.dt.float32
EXP = mybir.ActivationFunctionType.Exp
BF16 = mybir.dt.bfloat16

_NC = None


def _body(tc, xt_d, wq_d, wk_d, wv_d, mask_d, out_d):
    nc = tc.nc

    with contextlib.ExitStack() as ctx:
        per = ctx.enter_context(tc.tile_pool(name="per", bufs=1))
        # Persistent SBUF: QT/KT as [d_inner=128, d_tile, q]; V interleaved as
        # [k_inner, k_tile, half, 257] with a ones column at [..., 256].
        qt = per.tile([P, DT, S], BF16)
        kt = per.tile([P, DT, S], BF16)
        # V split into one tile per 4-k-tile group so phase-2 ctx matmuls on
        # early k-tiles never wait on late phase-1 V writes (coarse per-tile
        # dependency tracking).
        vg = [per.tile([P, 4, 2, HH + 1], BF16, name=f"v{g}") for g in range(4)]
        mask_sb = per.tile([P, 4, 512], BF16)

        # All pools live for the whole kernel: closing a pool at the phase
        # boundary inserts a drain barrier that stalls the first phase-2
        # matmul until every phase-1 consumer has completed. Instead one
        # PSUM pool carries explicit bank tags (q0-3/k0-3) that both phases
        # share; WAR hazards are then tracked per bank, not per pool.
        wpool = ctx.enter_context(tc.tile_pool(name="wpool", bufs=1))
        xpool = ctx.enter_context(tc.tile_pool(name="xpool", bufs=2))
        psu = ctx.enter_context(tc.tile_pool(name="psu", bufs=1, space="PSUM"))
        apool = ctx.enter_context(tc.tile_pool(name="apool", bufs=2))
        opool = ctx.enter_context(tc.tile_pool(name="opool", bufs=3))

        # ---------------- Phase 1: QT, KT, V projections ----------------
        if True:
            wq_sb = wpool.tile([P, EO, HD], BF16)
            wk_sb = wpool.tile([P, EO, HD], BF16)
            wv_sb = wpool.tile([P, EO, HD], BF16)
            xt_r = xt_d.rearrange("(eo p) q -> p eo q", p=P)
            wq_r = wq_d.rearrange("(eo p) d -> p eo d", p=P)
            wk_r = wk_d.rearrange("(eo p) d -> p eo d", p=P)
            wv_r = wv_d.rearrange("(eo p) d -> p eo d", p=P)

            # PE p-state warmup: the tensor engine runs at ~half clock until
            # it has executed ~3us continuously. Burn that ramp on junk
            # matmuls (uninitialized SBUF in, scratch psum out, never read)
            # DURING the initial DMA wait, so the first real matmuls run at
            # full speed. 7 x 512-wide ~= 3us at the ramp clock, ending just
            # before the first input chunk lands.
            warm = psu.tile([P, 512], F32, tag="k3", name="warm")
            for _ in range(7):
                nc.tensor.matmul(
                    warm, lhsT=qt[:, 0, 0:P], rhs=qt[:, 0, 0:512],
                    start=True, stop=True,
                )

            xc0 = xpool.tile([P, EO, 512], BF16, tag="xc", name="xc")
            # Head of the pipeline: wq/xc0 in interleaved 2-eo chunks. The
            # qc=0 projection loops run eo-OUTER with 4 concurrent psum
            # groups, so the first matmul needs only the first 2-eo chunk
            # (512KB) and consumption (0.85us/eo) stays below steady-state
            # DMA supply (~0.77us/eo). Later tensors have slack and go out
            # as big merged triggers.
            # Two parallel trigger streams: sync issues the weights, scalar
            # issues the x chunks, so the first wq and xc0 transfers start
            # simultaneously instead of serializing on one engine.
            # Finest chunks first (the first matmul needs only eo=0 of each),
            # then pairs -- supply rate stays ahead of the eo-outer consumers.
            head_chunks = [(0, 1), (1, 2), (2, 4), (4, 6), (6, 8)]
            for a, b in head_chunks:
                nc.sync.dma_start(
                    out=wq_sb[:, a:b, :], in_=wq_r[:, a:b, :]
                )
                nc.scalar.dma_start(
                    out=xc0[:, a:b, :], in_=xt_r[:, a:b, 0:512]
                )
            nc.sync.dma_start(out=wk_sb[:, 0:4, :], in_=wk_r[:, 0:4, :])
            nc.sync.dma_start(out=wk_sb[:, 4:8, :], in_=wk_r[:, 4:8, :])
            nc.sync.dma_start(out=wv_sb, in_=wv_r)
            nc.sync.dma_start(out=mask_sb, in_=mask_d.rearrange("r p c -> p r c"))
            # V's ones columns: emitted after the triggers so the gpsimd ops
            # don't delay the preamble handoff that gates the first trigger.
            for g in vg:
                nc.gpsimd.memset(g[:, :, :, HH], 1.0)

            for qc in range(4):  # 512-wide q/k chunk
                if qc == 0:
                    xc = xc0
                else:
                    xc = xpool.tile([P, EO, 512], BF16, tag="xc", name="xc")
                    eng = nc.scalar if qc == 1 else nc.sync
                    eng.dma_start(
                        out=xc, in_=xt_r[:, :, qc * 512 : (qc + 1) * 512]
                    )

                # QT / KT: out[d_tile, q-chunk] = sum_e W[e, d].T @ XT[e, q].
                # eo-outer: 4 dm accumulation groups open at once (4 banks),
                # each eo chunk is consumed as soon as it lands. PSUM->SBUF
                # casts alternate scalar/vector so neither serializes the
                # bank handoff to the next projection.
                for w_sb, dst, tg in ((wq_sb, qt, "q"), (wk_sb, kt, "k")):
                    pss4 = [
                        psu.tile([P, 512], F32, tag=f"{tg}{dm}", name=f"p{tg}{dm}")
                        for dm in range(DT)
                    ]
                    for eo in range(EO):
                        for dm in range(DT):
                            nc.tensor.matmul(
                                pss4[dm],
                                lhsT=w_sb[:, eo, dm * P : (dm + 1) * P],
                                rhs=xc[:, eo, :],
                                start=(eo == 0),
                                stop=(eo == EO - 1),
                            )
                    for dm in range(DT):
                        dslice = dst[:, dm, qc * 512 : (qc + 1) * 512]
                        if dm % 2 == 0:
                            nc.scalar.copy(dslice, pss4[dm])
                        else:
                            nc.vector.tensor_copy(dslice, pss4[dm])

                # V: out[k_tile, d] = sum_e XT[e, k].T @ Wv[e, d], stored as
                # two 256-wide halves (ones columns already set).
                for ki in range(4):
                    kg = qc * 4 + ki
                    ps = psu.tile([P, 512], F32, tag=f"q{ki}", name="psv")
                    for eo in range(EO):
                        nc.tensor.matmul(
                            ps,
                            lhsT=xc[:, eo, ki * P : (ki + 1) * P],
                            rhs=wv_sb[:, eo, :],
                            start=(eo == 0),
                            stop=(eo == EO - 1),
                        )
                    nc.vector.tensor_copy(vg[qc][:, ki, 0, 0:HH], ps[:, 0:HH])
                    nc.vector.tensor_copy(vg[qc][:, ki, 1, 0:HH], ps[:, HH:HD])

        # ---------------- Phase 2: attention ----------------
        # Scores rotate through KT's four banks (k0-3, drained earliest in
        # phase 1); ctx accumulators take QT/V's banks (q0-3).
        if True:
            def emit_scores(M):
                # One at tile per k-tile so a ctx matmul on k-tile j waits
                # only for j's own exp+mask, not the whole super-tile's.
                njt = 4 * M + 4  # causal: k-tiles 0 .. 4M+3
                at = [
                    apool.tile([P, 512], BF16, tag=f"at{j}", name=f"at{j}")
                    for j in range(njt)
                ]
                for j in range(njt):
                    r = j - 4 * M
                    # Diagonal-zone tiles: cols < 128r are fully masked; bf16
                    # matmuls have no minimum-width penalty, so trim exactly.
                    off = P * r if r > 0 else 0
                    ps = psu.tile([P, 512], F32, tag=f"k{j % 4}", name="ps_s")
                    for dt_i in range(DT):
                        nc.tensor.matmul(
                            ps[:, off:512],
                            lhsT=kt[:, dt_i, j * P : (j + 1) * P],
                            rhs=qt[:, dt_i, M * 512 + off : (M + 1) * 512],
                            start=(dt_i == 0),
                            stop=(dt_i == DT - 1),
                        )
                    a_j = at[j][:, off:512]
                    # attnT[k, q] = exp(scoresT / sqrt(hd)); masking after.
                    nc.scalar.activation(a_j, ps[:, off:512], EXP, scale=SCALE)
                    if r >= 0:  # diagonal-zone: zero invalid (q < k) cols
                        nc.vector.tensor_mul(a_j, a_j, mask_sb[:, r, off:512])
                return at

            def emit_ctx(M, at, final=False):
                # ctx[q_sub, d] += attnT_tile.T @ V-half (column 256 of each
                # psum accumulates the softmax denominator via V's ones
                # column). NOTE: hardware start=True invalidates has_written
                # for the WHOLE psum bank, so only one accumulation group may
                # be open per bank at a time; 4 banks = 2 subtiles in flight,
                # so process s in two waves, reusing bank tags across waves.
                for wave in ((3, 2), (1, 0)):
                    wtiles = {}
                    for wi, s in enumerate(wave):
                        cps = [
                            psu.tile(
                                [P, 512], F32, tag=f"q{2 * wi + h}", name=f"c{wi}{h}"
                            )
                            for h in (0, 1)
                        ]
                        wtiles[s] = cps
                        nj = 4 * M + s + 1
                        if final and s == 0:
                            # Very last chain: run h0 (which carries the
                            # denominator column) to completion FIRST, so its
                            # reciprocal + scale + store overlap the h1 chain
                            # and only h1's short store tail remains exposed.
                            for h in (0, 1):
                                for j in range(nj):
                                    nc.tensor.matmul(
                                        cps[h][:, 0 : HH + 1],
                                        lhsT=at[j][:, s * P : (s + 1) * P],
                                        rhs=vg[j // 4][:, j % 4, h, :],
                                        start=(j == 0),
                                        stop=(j == nj - 1),
                                    )
                                if h == 0:
                                    rinv0 = opool.tile([P, 1], F32, tag="rinv")
                                    nc.vector.reciprocal(
                                        rinv0, cps[0][:, HH : HH + 1]
                                    )
                                    o0 = opool.tile([P, HH], BF16, tag="o0", name="o0")
                                    nc.scalar.mul(o0, cps[0][:, 0:HH], rinv0)
                                    row0 = M * 512
                                    nc.sync.dma_start(
                                        out=out_d[row0 : row0 + P, 0:HH], in_=o0
                                    )
                            # h1 scale AND trigger both on scalar: no cross-
                            # engine semaphore hop on the very last store.
                            o1 = opool.tile([P, HH], BF16, tag="o1", name="o1")
                            nc.scalar.mul(o1, cps[1][:, 0:HH], rinv0)
                            nc.scalar.dma_start(
                                out=out_d[row0 : row0 + P, HH:HD], in_=o1
                            )
                            continue
                        for j in range(nj):
                            a_st = at[j][:, s * P : (s + 1) * P]
                            for h in (0, 1):
                                nc.tensor.matmul(
                                    cps[h][:, 0 : HH + 1],
                                    lhsT=a_st,
                                    rhs=vg[j // 4][:, j % 4, h, :],
                                    start=(j == 0),
                                    stop=(j == nj - 1),
                                )
                    for s in wave:
                        if final and s == 0:
                            continue
                        cps = wtiles[s]
                        rinv = opool.tile([P, 1], F32, tag="rinv")
                        nc.vector.reciprocal(rinv, cps[0][:, HH : HH + 1])
                        row0 = M * 512 + s * P
                        # Per-half output on two engines in parallel (scalar
                        # scales+triggers h0, vector h1) so the final store
                        # chain after the last matmul is as short as possible.
                        nq = 2
                        w = HD // nq
                        for qi in range(nq):
                            # bf16 store: halves DVE scale time and DMA bytes;
                            # host upcasts. Error budget absorbs it (~0.4%).
                            oq = opool.tile([P, w], BF16, tag=f"o{qi}", name="oq")
                            c0 = qi * w
                            src_ap = cps[c0 // HH]
                            lo = c0 % HH
                            if qi % 2 == 0:
                                nc.scalar.mul(oq, src_ap[:, lo : lo + w], rinv)
                                nc.sync.dma_start(
                                    out=out_d[row0 : row0 + P, c0 : c0 + w], in_=oq
                                )
                            else:
                                nc.vector.tensor_scalar_mul(
                                    oq, src_ap[:, lo : lo + w], rinv
                                )
                                nc.scalar.dma_start(
                                    out=out_d[row0 : row0 + P, c0 : c0 + w], in_=oq
                                )

            # Software-pipeline: emit scores(M+1) before ctx(M) so every ctx
            # chain has a full super-tile of scores runway between a diagonal
            # tile's exp+mask and the ctx matmuls that consume it (at M=0 the
            # scores alone are too short to hide the scalar/vector latency).
            # at-tile tags alternate between 2 buffers, which exactly matches
            # this interleave depth.
            prev = None
            for M in range(NSUP):  # q super-tile: q in [512M, 512(M+1))
                at = emit_scores(M)
                if prev is not None:
                    emit_ctx(M - 1, prev)
                prev = at
            emit_ctx(NSUP - 1, prev, final=True)


def _build_nc():
    nc = bacc.Bacc("TRN2", target_bir_lowering=False, debug=False, num_devices=8)
    xt_d = nc.dram_tensor("xt", [E, S], BF16, kind="ExternalInput")
    wq_d = nc.dram_tensor("wq", [E, HD], BF16, kind="ExternalInput")
    wk_d = nc.dram_tensor("wk", [E, HD], BF16, kind="ExternalInput")
    wv_d = nc.dram_tensor("wv", [E, HD], BF16, kind="ExternalInput")
    mask_d = nc.dram_tensor("mask", [4, P, 512], BF16, kind="ExternalInput")
    out_d = nc.dram_tensor("out", [S, HD], BF16, kind="ExternalOutput")
    with tile.TileContext(nc) as tc:
        _body(tc, xt_d.ap(), wq_d.ap(), wk_d.ap(), wv_d.ap(), mask_d.ap(), out_d.ap())
    nc.compile()
    return nc


def _mask_np():
    # mask[r][k_local, q_local] = 1 iff q_local >= 128*r + k_local
    q = np.arange(512)[None, :]
    k = np.arange(P)[:, None]
    return np.stack(
        [(q >= (P * r + k)).astype(np.float32) for r in range(4)], axis=0
    ).astype(ml_dtypes.bfloat16)


def _in_maps(embedded, Wq, Wk, Wv):
    embedded = np.asarray(embedded, dtype=np.float32)
    Wq = np.asarray(Wq, dtype=np.float32)
    Wk = np.asarray(Wk, dtype=np.float32)
    Wv = np.asarray(Wv, dtype=np.float32)
    mask = _mask_np()
    in_maps = []
    for core in range(8):
        b, h = divmod(core, 2)
        in_maps.append(
            {
                "xt": np.ascontiguousarray(embedded[b].T).astype(ml_dtypes.bfloat16),
                "wq": np.ascontiguousarray(Wq[h]).astype(ml_dtypes.bfloat16),
                "wk": np.ascontiguousarray(Wk[h]).astype(ml_dtypes.bfloat16),
                "wv": np.ascontiguousarray(Wv[h]).astype(ml_dtypes.bfloat16),
                "mask": mask,
            }
        )
    return in_maps


def _gather(results):
    out = np.empty((B, S, H * HD), np.float32)
    for core in range(8):
        b, h = divmod(core, 2)
        out[b, :, h * HD : (h + 1) * HD] = results[core]["out"].astype(np.float32)
    return out


def _get_nc():
    global _NC
    if _NC is None:
        _NC = _build_nc()
    return _NC


def kernel(embedded, Wq, Wk, Wv):
    res = bass_utils.run_bass_kernel_spmd(
        _get_nc(), _in_maps(embedded, Wq, Wk, Wv), core_ids=list(range(8))
    )
    return _gather(res.results)


def kernel_traced(embedded, Wq, Wk, Wv):
    """Like kernel() but with NTFF tracing; returns (out, BassKernelResults)."""
    res = bass_utils.run_bass_kernel_spmd(
        _get_nc(), _in_maps(embedded, Wq, Wk, Wv), core_ids=list(range(8)), trace=True
    )
    return _gather(res.results), res
